# revision 29
# baseline (speedup 1.0000x reference)
"""Trainium2 Bass kernel for nn_CausalPrefixAttention (8-core SPMD), v3.

Sharding: core = b*4 + hg  (b in 0..1 batch, hg in 0..3 head-group of 2 heads).
Data parallel over batch, tensor parallel over heads: each core gets
Wq/Wkv column slices and Wo row slices for its 2 heads, computes its partial
out-projection [1024, 1024] in bf16; host sums the 4 partials per batch in
f32 and adds bo.

Key points (152us baseline -> this):
  - All activations/weights shipped in bf16 (PE matmul rate is 1 cycle/row
    for both f32r and bf16; DMA halves).
  - x^T / cx^T pre-transposed on the HOST -> zero PE transposes.
  - LayerNorm stats on host; rank-2 augmented contraction rows (-mu, std)
    against (u=colsum(W'), b=beta@W); per-token rs=1/std fused into the
    PSUM->SBUF copies (tensor_tensor by an rs-broadcast tile for q^T/k^T,
    tensor_scalar by an rs-column for v natural).
  - v projected directly in natural [token, feature] layout with a ones
    column so the softmax denominator falls out of PV row 64.
  - Both heads' sim live in one 2-bank PSUM tile -> ONE exp instruction
    per 128-key tile (halves ACT instruction overhead).
  - Schedule: input projections (DMA-paced) -> v chains -> g0-input
    attention starts while context DMA still streaming; g1-input sims+exp
    run during the context-projection window (PVs deferred via stored p);
    g0 out-projection interleaves into g1's attention loop.
  - ACT does exp + half the output copies; DVE everything else; SP issues
    all DMAs; PE continuously busy to hold the 2.4GHz pstate.
"""

import os
import sys

for _p in ("/opt/trn_rl_repo", "/root/.axon_site/_ro/trn_rl_repo"):
    if os.path.isdir(_p) and _p not in sys.path:
        sys.path.append(_p)

import numpy as np
import ml_dtypes

import concourse.mybir as mybir
import concourse.tile as tile
from concourse import bacc
from concourse.bass_utils import run_bass_kernel_spmd

F32 = mybir.dt.float32
BF16 = mybir.dt.bfloat16
AF = mybir.ActivationFunctionType
ALU = mybir.AluOpType
BF16_NP = ml_dtypes.bfloat16

B, N, M, DIM, INNER, HEADS, DH = 2, 1024, 1024, 1024, 512, 8, 64
EPS = 1e-5
NT = N // 128      # token tiles per batch (8)
KC = DIM // 128    # contraction chunks (8)


def build_program(unroll=1, phase=2):
    nc = bacc.Bacc("TRN2", target_bir_lowering=False, debug=False)

    xt_d = nc.dram_tensor("xt", [128, KC, N], BF16, kind="ExternalInput")
    cxt_d = nc.dram_tensor("cxt", [128, KC, M], BF16, kind="ExternalInput")
    win_d = nc.dram_tensor("win", [128, KC + 1, 384], BF16, kind="ExternalInput")
    wcx_d = nc.dram_tensor("wcx", [128, KC, 256], BF16, kind="ExternalInput")
    wo_d = nc.dram_tensor("wo", [128, DIM], BF16, kind="ExternalInput")
    st_d = nc.dram_tensor("st", [2, N], BF16, kind="ExternalInput")
    rsb_d = nc.dram_tensor("rsb", [128, N], BF16, kind="ExternalInput")
    rsc_d = nc.dram_tensor("rsc", [128, NT], F32, kind="ExternalInput")
    sel_d = nc.dram_tensor("sel", [1, 256], BF16, kind="ExternalInput")
    tri_d = nc.dram_tensor("tri", [128, 128], BF16, kind="ExternalInput")
    o_d = nc.dram_tensor("o", [N, DIM], BF16, kind="ExternalOutput")

    with tile.TileContext(nc) as tc:
        for _ in range(unroll):
            _emit(nc, tc, xt_d, cxt_d, win_d, wcx_d, wo_d, st_d, rsb_d,
                  rsc_d, sel_d, tri_d, o_d, phase)
    nc.compile()
    return nc


def _emit(nc, tc, xt_d, cxt_d, win_d, wcx_d, wo_d, st_d, rsb_d, rsc_d,
          sel_d, tri_d, o_d, phase=2):
    from contextlib import ExitStack

    ctx = ExitStack()
    with ctx:
        consts = ctx.enter_context(tc.tile_pool(name="consts", bufs=1))
        wpool = ctx.enter_context(tc.tile_pool(name="wpool", bufs=1))
        projp = ctx.enter_context(tc.tile_pool(name="projp", bufs=3))
        vnp = ctx.enter_context(tc.tile_pool(name="vnp", bufs=16))
        ppool = ctx.enter_context(tc.tile_pool(name="ppool", bufs=2))
        pg1p = ctx.enter_context(tc.tile_pool(name="pg1p", bufs=8))
        # xt/cxt (32KB/part) live only through the projection+sim phase;
        # their scope closes before the output-staging pools open.
        xctx = ExitStack()
        xpool = xctx.enter_context(tc.tile_pool(name="xpool", bufs=1))

        # preheat the ACT exp table during the DMA dead time
        preheat = consts.tile([1, 8], BF16)
        nc.vector.memset(preheat, 0.0)
        nc.scalar.activation(out=preheat, in_=preheat, func=AF.Exp)

        # ---- input DMAs, all on SP (sync), in consumption order; weight
        # chunks batched in pairs to amortize per-DMA overhead ----
        win = wpool.tile([128, KC + 1, 384], BF16, tag="win")
        xt = xpool.tile([128, KC, N], BF16, tag="xt")
        st = consts.tile([2, N], BF16)
        rsc = consts.tile([128, NT], F32)
        rsb = consts.tile([128, N], BF16)
        for c in range(KC):
            if c % 2 == 0:
                hi = KC + 1 if c == 6 else c + 2
                nc.sync.dma_start(out=win[:, c:hi, :], in_=win_d[:, c:hi, :])
            nc.sync.dma_start(out=xt[:, c:c + 1, :], in_=xt_d[:, c:c + 1, :])
            if c == 2:
                nc.sync.dma_start(out=st, in_=st_d[:])
                nc.sync.dma_start(out=rsc, in_=rsc_d[:])
        nc.sync.dma_start(out=rsb, in_=rsb_d[:])
        tri = consts.tile([128, 128], BF16)
        nc.sync.dma_start(out=tri, in_=tri_d[:])
        sel2 = consts.tile([1, 256], BF16)
        nc.sync.dma_start(out=sel2, in_=sel_d[:])
        wcx = wpool.tile([128, KC, 256], BF16, tag="wcx")
        cxt = xpool.tile([128, KC, M], BF16, tag="cxt")
        for c in range(KC):
            if c % 4 == 0:
                nc.sync.dma_start(out=wcx[:, c:c + 4, :],
                                  in_=wcx_d[:, c:c + 4, :])
            nc.sync.dma_start(out=cxt[:, c:c + 1, :], in_=cxt_d[:, c:c + 1, :])
        wo = wpool.tile([128, DIM], BF16, tag="wo")
        nc.sync.dma_start(out=wo, in_=wo_d[:])

        qT = projp.tile([128, N], BF16, tag="proj", name="qT")
        kinT = projp.tile([128, N], BF16, tag="proj", name="kinT")
        kcxT = projp.tile([128, M], BF16, tag="proj", name="kcxT")
        vn = [None] * 16

        # ---- q/k projections (own PSUM scope: 4 banks) ----
        with tc.tile_pool(name="psQ", bufs=1, space="PSUM") as psQ, \
             tc.tile_pool(name="psK", bufs=1, space="PSUM") as psK:
            q_ps = [psQ.tile([128, 512], F32, tag=f"A{g}", name=f"qps{g}")
                    for g in range(2)]
            k_ps = [psK.tile([128, 512], F32, tag=f"A{2 + g}", name=f"kps{g}")
                    for g in range(2)]
            for c in range(KC):
                for g in range(2):
                    sp = slice(g * 512, (g + 1) * 512)
                    nc.tensor.matmul(q_ps[g], win[:, c, 0:128], xt[:, c, sp],
                                     start=(c == 0), stop=False)
                for g in range(2):
                    sp = slice(g * 512, (g + 1) * 512)
                    nc.tensor.matmul(k_ps[g], win[:, c, 128:256], xt[:, c, sp],
                                     start=(c == 0), stop=False)
            for g in range(2):
                sp = slice(g * 512, (g + 1) * 512)
                nc.tensor.matmul(q_ps[g], win[0:2, KC, 0:128], st[:, sp],
                                 start=False, stop=True)
                nc.tensor.matmul(k_ps[g], win[0:2, KC, 128:256], st[:, sp],
                                 start=False, stop=True)
            # copy order k_g0, q_g0 first: g0-input sims only need the
            # g0 halves, and early psK release unblocks the v chains.
            with nc.allow_low_precision(reason="bf16 activations"):
                for dst, src_ps, g in ((kinT, k_ps, 0), (qT, q_ps, 0),
                                       (kinT, k_ps, 1), (qT, q_ps, 1)):
                    sp = slice(g * 512, (g + 1) * 512)
                    nc.vector.tensor_tensor(
                        out=dst[:, sp], in0=src_ps[g], in1=rsb[:, sp],
                        op=ALU.mult)

        # ---- attention pools (sim 2x2 banks + o 2 banks) ----
        with tc.tile_pool(name="psS", bufs=1, space="PSUM") as psS, \
             tc.tile_pool(name="psO", bufs=1, space="PSUM") as psO:

            def v_chain(t, src, wsrc, wsl_v, aug, scale, psV):
                v_ps = psV.tile([128, 512], F32, tag=f"V{t % 2}",
                                name=f"vps{t}")
                for c in range(KC):
                    nc.tensor.matmul(
                        v_ps[:, 0:128], src[:, c, t * 128:(t + 1) * 128],
                        wsrc[:, c, wsl_v],
                        start=(c == 0), stop=(not aug and c == KC - 1))
                if aug:
                    nc.tensor.matmul(v_ps[:, 0:128],
                                     st[:, t * 128:(t + 1) * 128],
                                     win[0:2, KC, 256:384],
                                     start=False, stop=True)
                base = 8 if aug else 0
                v_t = vnp.tile([128, 130], BF16, tag="vn",
                               name=f"vn{base + t}")
                vn[base + t] = v_t
                nc.vector.memset(
                    v_t.rearrange("p (a b) -> p a b", b=65)[:, :, 64:65], 1.0)
                vv = v_t.rearrange("p (a b) -> p a b", b=65)[:, :, 0:64]
                vs = v_ps[:, 0:128].rearrange("p (a b) -> p a b", b=64)
                with nc.allow_low_precision(reason="bf16 activations"):
                    if scale:
                        nc.vector.tensor_scalar(
                            out=vv, in0=vs, scalar1=rsc[:, t:t + 1],
                            scalar2=None, op0=ALU.mult)
                    else:
                        nc.vector.tensor_copy(out=vv, in_=vs)

            # one merged sim tile (2 banks) per j-tile; p bf16 [128, 1024]
            def emit_sim(g, src, j, off):
                kT = kinT if src == "in" else kcxT
                ps = psS.tile([128, 1024], F32, tag="sim", bufs=2, name="sim")
                for h in (0, 1):
                    hsl = slice(64 * h, 64 * h + 64)
                    nc.tensor.matmul(
                        ps[:, 512 * h + off:512 * h + 512],
                        kT[hsl, j * 128:(j + 1) * 128],
                        qT[hsl, g * 512 + off:(g + 1) * 512],
                        start=True, stop=True)
                return ps

            def emit_exp(ps, off, diag, pool, name="p"):
                p_t = pool.tile([128, 1024], BF16, tag=name, name=name)
                if off:
                    pv = p_t.rearrange("p (a b) -> p a b", b=512)[:, :, off:]
                    sv = ps.rearrange("p (a b) -> p a b", b=512)[:, :, off:]
                else:
                    pv, sv = p_t, ps
                nc.scalar.activation(out=pv, in_=sv, func=AF.Exp)
                if diag:
                    for h in (0, 1):
                        nc.vector.tensor_tensor(
                            out=p_t[:, 512 * h + off:512 * h + off + 128],
                            in0=p_t[:, 512 * h + off:512 * h + off + 128],
                            in1=tri, op=ALU.mult)
                return p_t

            def emit_pv(o_ps, p_t, jg, off, start, stop):
                for h in (0, 1):
                    nc.tensor.matmul(
                        o_ps[h][0:65, off:512],
                        vn[jg][:, 65 * h:65 * h + 65],
                        p_t[:, 512 * h + off:512 * h + 512],
                        start=start, stop=stop)

            # ---- v chains + g0-input attention + cx proj (+g1-in sims) ----
            o_g0 = [psO.tile([128, 512], F32, tag=f"o{h}", name=f"o0_{h}")
                    for h in (0, 1)]
            with tc.tile_pool(name="psV", bufs=1, space="PSUM") as psV:
                for t in range(NT):
                    v_chain(t, xt, win, slice(256, 384), True, True, psV)

                # g0-input attention: j=0..3, off=128j, all diagonal
                g0_in = []
                for j in range(4):
                    g0_in.append(emit_sim(0, "in", j, 128 * j))
                for j in range(4):
                    p_t = emit_exp(g0_in[j], 128 * j, True, ppool)
                    emit_pv(o_g0, p_t, 8 + j, 128 * j,
                            start=(j == 0), stop=False)
                    g0_in[j] = None

                # kcx (DMA-paced) interleaved with g1-input sims (PV deferred)
                kc_ps = [psV.tile([128, 512], F32, tag=f"V{g}",
                                  name=f"kcps{g}") for g in range(2)]
                pg1 = [None] * NT
                for c in range(KC):
                    for g in range(2):
                        sp = slice(g * 512, (g + 1) * 512)
                        nc.tensor.matmul(kc_ps[g], wcx[:, c, 0:128],
                                         cxt[:, c, sp],
                                         start=(c == 0), stop=(c == KC - 1))
                    off = max(0, 128 * (c - 4))
                    ps = emit_sim(1, "in", c, off)
                    pg1[c] = emit_exp(ps, off, c >= 4, pg1p, name=f"pg1_{c}")
                with nc.allow_low_precision(reason="bf16 activations"):
                    for g in range(2):
                        sp = slice(g * 512, (g + 1) * 512)
                        nc.vector.tensor_copy(out=kcxT[:, sp], in_=kc_ps[g])
                for t in range(NT):
                    v_chain(t, cxt, wcx, slice(128, 256), False, False, psV)

            xctx.close()
            p2ctx = ExitStack()
            psO2 = p2ctx.enter_context(
                tc.tile_pool(name="psO2", bufs=1, space="PSUM"))

            if phase == 1:
                for t, src_t in enumerate((qT, kinT, kcxT)):
                    nc.sync.dma_start(
                        out=o_d[t * 128:(t + 1) * 128, :], in_=src_t)
                return

            o_r = o_d.rearrange("(t p) d -> p t d", p=128)
            otp = ctx.enter_context(tc.tile_pool(name="otp", bufs=2))
            ostp = ctx.enter_context(tc.tile_pool(name="ostp", bufs=3))
            tiny = ctx.enter_context(tc.tile_pool(name="tiny", bufs=8))

            def finalize_pre(g, o_ps):
                """l row copies (DVE h0 / ACT h1) - pre-PE part."""
                lrec = [tiny.tile([1, 512], BF16, tag=f"lr{g}{h}", bufs=1,
                                  name=f"lr{h}") for h in (0, 1)]
                with nc.allow_low_precision(reason="l in bf16"):
                    nc.vector.tensor_copy(out=lrec[0], in_=o_ps[0][64:65, :])
                    nc.scalar.copy(out=lrec[1], in_=o_ps[1][64:65, :])
                return lrec

            def finalize_post(g, o_ps, lrec):
                """sel-broadcast of l (PE), reciprocal on the broadcast,
                oT extraction (DVE)."""
                lbc_ps = psS.tile([128, 1024], F32, tag="sim", bufs=2,
                                  name="lbc")
                for h in (0, 1):
                    nc.tensor.matmul(lbc_ps[:, 0:512],
                                     sel2[:, 128 * h:128 * h + 128],
                                     lrec[h], start=(h == 0), stop=(h == 1))
                lbc = tiny.tile([128, 512], F32, tag="lbc", bufs=2,
                                name="lbc")
                nc.vector.reciprocal(out=lbc, in_=lbc_ps[:, 0:512])
                oT = otp.tile([128, 512], BF16, tag="oT")
                with nc.allow_low_precision(reason="bf16 attn out"):
                    for h in (0, 1):
                        nc.vector.tensor_tensor(
                            out=oT[64 * h:64 * h + 64, :],
                            in0=o_ps[h][0:64, :],
                            in1=lbc[64 * h:64 * h + 64, :], op=ALU.mult)
                return oT

            def outproj_tile(g, t, oT, mode):
                """[128tok, 1024] out-projection for one token tile. The
                two fin halves reuse o_g0's banks (dead after its oT is
                extracted), keeping the sim rotation pure. mode: 'dve'
                (in-loop; ACT busy with exp) or 'act' (endgame; ACT idle)."""
                fps = []
                for half in (0, 1):
                    fp = psO.tile([128, 512], F32, tag=f"o{half}",
                                  name=f"fin{half}")
                    nc.tensor.matmul(fp, oT[:, t * 128:(t + 1) * 128],
                                     wo[:, half * 512:half * 512 + 512],
                                     start=True, stop=True)
                    fps.append(fp)
                ost = ostp.tile([128, 1, DIM], BF16, tag="ost", name="ost")
                with nc.allow_low_precision(reason="bf16 out"):
                    for half in (0, 1):
                        dst = ost[:, 0, half * 512:half * 512 + 512]
                        if mode == "act":
                            nc.scalar.copy(out=dst, in_=fps[half])
                        else:
                            nc.vector.tensor_copy(out=dst, in_=fps[half])
                nc.sync.dma_start(out=o_r[:, g * 4 + t:g * 4 + t + 1, :],
                                  in_=ost)

            # ---- g0 context attention ----
            sims = [None, None]
            sims[0] = emit_sim(0, "cx", 0, 0)
            for j in range(KC):
                if j + 1 < KC:
                    sims[(j + 1) % 2] = emit_sim(0, "cx", j + 1, 0)
                p_t = emit_exp(sims[j % 2], 0, False, ppool)
                emit_pv(o_g0, p_t, j, 0, start=False, stop=(j == KC - 1))
                sims[j % 2] = None

            # ---- g0 l-row copies, then g1's deferred input PVs (fresh
            # o banks o2/o3 so nothing waits on g0's release) ----
            lrec0 = finalize_pre(0, o_g0)
            o_g1 = [psO2.tile([128, 512], F32, tag=f"o{2 + h}",
                              name=f"o1_{h}") for h in (0, 1)]
            for j in range(NT):
                off = max(0, 128 * (j - 4))
                emit_pv(o_g1, pg1[j], 8 + j, off, start=(j == 0), stop=False)
                pg1[j] = None
            oT0 = finalize_post(0, o_g0, lrec0)

            # ---- g1 context attention with g0 out-proj interleaved ----
            oji = [0]

            def drain_outproj(k, oT_):
                for _ in range(k):
                    if oji[0] < 4:
                        outproj_tile(0, oji[0], oT_, mode="dve")
                        oji[0] += 1

            sims[0] = emit_sim(1, "cx", 0, 0)
            for j in range(KC):
                if j + 1 < KC:
                    sims[(j + 1) % 2] = emit_sim(1, "cx", j + 1, 0)
                p_t = emit_exp(sims[j % 2], 0, False, ppool)
                emit_pv(o_g1, p_t, j, 0, start=False, stop=(j == KC - 1))
                sims[j % 2] = None
                if j >= 1:
                    drain_outproj(1, oT0)
            drain_outproj(4, oT0)

            # ---- g1 finalize + out-proj, fine-grained by token block so
            # the first out-projection starts right after l lands ----
            lrec1 = finalize_pre(1, o_g1)
            lbc_ps = psS.tile([128, 1024], F32, tag="sim", bufs=2,
                              name="lbc1")
            for h in (0, 1):
                nc.tensor.matmul(lbc_ps[:, 0:512],
                                 sel2[:, 128 * h:128 * h + 128],
                                 lrec1[h], start=(h == 0), stop=(h == 1))
            lbc1 = tiny.tile([128, 512], F32, tag="lbc", bufs=2, name="lbc1")
            oT1 = otp.tile([128, 512], BF16, tag="oT")
            for t in range(4):
                bsl = slice(t * 128, (t + 1) * 128)
                nc.vector.reciprocal(out=lbc1[:, bsl], in_=lbc_ps[:, bsl])
                with nc.allow_low_precision(reason="bf16 attn out"):
                    for h in (0, 1):
                        nc.vector.tensor_tensor(
                            out=oT1[64 * h:64 * h + 64, bsl],
                            in0=o_g1[h][0:64, bsl],
                            in1=lbc1[64 * h:64 * h + 64, bsl], op=ALU.mult)
                outproj_tile(1, t, oT1, mode="act")
            p2ctx.close()


_NC_CACHE = None


def _get_nc():
    global _NC_CACHE
    if _NC_CACHE is None:
        _NC_CACHE = build_program()
    return _NC_CACHE


def make_in_maps(x, context, gamma, beta, Wq, Wkv, Wo, bo):
    x = np.asarray(x, np.float32)
    context = np.asarray(context, np.float32)
    gamma = np.asarray(gamma, np.float32)
    beta = np.asarray(beta, np.float32)
    Wq = np.asarray(Wq, np.float32)
    Wkv = np.asarray(Wkv, np.float32)
    Wo = np.asarray(Wo, np.float32)

    s = DH ** -0.5
    mu = x.mean(-1)                                   # [B, N]
    var = x.var(-1)
    std = np.sqrt(var + EPS)
    rs = 1.0 / std

    def tr(a):
        return np.ascontiguousarray(
            a.T.reshape(KC, 128, -1).transpose(1, 0, 2)).astype(BF16_NP)

    xts = [tr(x[b]) for b in range(B)]
    cxts = [tr(context[b]) for b in range(B)]

    in_maps = []
    for core in range(8):
        b, hg = divmod(core, 4)
        cols = slice(128 * hg, 128 * hg + 128)
        wq = Wq[:, cols] * gamma[:, None] * s
        uq = wq.sum(0)
        bq = beta @ Wq[:, cols] * s
        wk = Wkv[:, :INNER][:, cols] * gamma[:, None]
        uk = wk.sum(0)
        bk = beta @ Wkv[:, :INNER][:, cols]
        wv = Wkv[:, INNER:][:, cols] * gamma[:, None]
        uv = wv.sum(0)
        bv = beta @ Wkv[:, INNER:][:, cols]

        win = np.zeros((128, KC + 1, 384), np.float32)
        for c in range(KC):
            rows = slice(128 * c, 128 * c + 128)
            win[:, c, 0:128] = wq[rows]
            win[:, c, 128:256] = wk[rows]
            win[:, c, 256:384] = wv[rows]
        win[0, KC, 0:128] = uq
        win[1, KC, 0:128] = bq
        win[0, KC, 128:256] = uk
        win[1, KC, 128:256] = bk
        win[0, KC, 256:384] = uv
        win[1, KC, 256:384] = bv

        wcx = np.zeros((128, KC, 256), np.float32)
        for c in range(KC):
            rows = slice(128 * c, 128 * c + 128)
            wcx[:, c, 0:128] = Wkv[:, :INNER][rows, cols]
            wcx[:, c, 128:256] = Wkv[:, INNER:][rows, cols]

        st = np.stack([-mu[b], std[b]]).astype(BF16_NP)
        rsb = np.ascontiguousarray(
            np.broadcast_to(rs[b][None, :], (128, N))).astype(BF16_NP)
        rsc = np.ascontiguousarray(
            rs[b].reshape(NT, 128).T).astype(np.float32)

        sel = np.zeros((1, 256), np.float32)
        sel[0, 0:64] = 1.0
        sel[0, 192:256] = 1.0
        tri = np.tril(np.ones((128, 128), np.float32)).T
        in_maps.append({
            "sel": sel.astype(BF16_NP),
            "tri": tri.astype(BF16_NP),
            "xt": xts[b],
            "cxt": cxts[b],
            "win": win.astype(BF16_NP),
            "wcx": wcx.astype(BF16_NP),
            "wo": np.ascontiguousarray(Wo[cols, :]).astype(BF16_NP),
            "st": st,
            "rsb": rsb,
            "rsc": rsc,
        })
    return in_maps


def assemble(results, bo):
    bo = np.asarray(bo, np.float32)
    out = np.zeros((B, N, DIM), np.float32)
    for core in range(8):
        b = core // 4
        out[b] += results[core]["o"].astype(np.float32)
    out += bo[None, None, :]
    return out


def kernel(x, context, gamma, beta, Wq, Wkv, Wo, bo):
    nc = _get_nc()
    in_maps = make_in_maps(x, context, gamma, beta, Wq, Wkv, Wo, bo)
    res = run_bass_kernel_spmd(nc, in_maps, list(range(8)))
    return assemble(res.results, bo)
